# revision 6
# baseline (speedup 1.0000x reference)
"""Squared Euclidean distance matrix kernel for Trainium2 (Bass/Tile).

out[i, j] = ||mat_1[i]||^2 + ||mat_2[j]||^2 - 2 * mat_1[i] . mat_2[j]

Sharding: mat_1 rows (and hence output rows) split across 8 NeuronCores;
mat_2 replicated.  Each core computes a (2048, 8192) tile independently.

Per-core plan:
  - Bt[k]   = -2 * B^T chunk k      (128 x 8192, k-chunk of the d=256 axis),
              built with PE transposes + DVE scale-copy.
  - sqb     = ||B_j||^2 broadcast to all 128 partitions (128 x 8192), built
              with an all-0.25 stationary matmul over Bt^2 (Bt holds -2B so
              sum(0.25 * (-2B)^2) = sum(B^2)).
  - per m-tile: sqa_col = rowwise ||A_i||^2 (128 x 1) via DVE square+reduce;
              At chunks via PE transpose.
  - main:   psum = At^T @ Bt  (fp32r matmuls, = -2 A.B^T tile)
            out  = (psum + sqa_col) + sqb      (single fused DVE op)
            DMA out 2MB contiguous slabs.
"""

import numpy as np

import concourse.bass as bass
import concourse.mybir as mybir
from concourse import bacc
from contextlib import ExitStack
from concourse.tile import TileContext
from concourse.masks import make_identity

F32 = mybir.dt.float32
F32R = mybir.dt.float32r
AX = mybir.AxisListType
OP = mybir.AluOpType

N_CORES = 8
M_FULL, N_FULL, D_FULL = 16384, 8192, 256


def build(m_sh=M_FULL // N_CORES, n=N_FULL, d=D_FULL):
    P = 128
    FD = 512                      # matmul moving free dim (1 PSUM bank fp32)
    KC = d // P                   # contraction chunks
    MT = m_sh // P                # m tiles per core
    NS = n // FD                  # n slices
    NB = n // P                   # b row tiles
    out_w = min(4096, n)          # out staging width
    OH = n // out_w
    SPW = out_w // FD             # slices per staging buffer

    nc = bacc.Bacc()
    a = nc.dram_tensor("a", [m_sh, d], F32, kind="ExternalInput")
    b = nc.dram_tensor("b", [n, d], F32, kind="ExternalInput")
    o = nc.dram_tensor("out", [m_sh, n], F32, kind="ExternalOutput")

    with ExitStack() as ctx:
        tc = ctx.enter_context(TileContext(nc))
        singles = ctx.enter_context(tc.tile_pool(name="singles", bufs=1))
        persist = ctx.enter_context(tc.tile_pool(name="persist", bufs=1))
        natp = ctx.enter_context(tc.tile_pool(name="natp", bufs=4))
        tmpp = ctx.enter_context(tc.tile_pool(name="tmpp", bufs=3))
        atp = ctx.enter_context(tc.tile_pool(name="atp", bufs=4))
        sqap = ctx.enter_context(tc.tile_pool(name="sqap", bufs=3))
        outp = ctx.enter_context(tc.tile_pool(name="outp", bufs=2))
        tpp = ctx.enter_context(tc.tile_pool(name="tpp", bufs=2, space="PSUM"))
        mmp = ctx.enter_context(tc.tile_pool(name="mmp", bufs=6, space="PSUM"))

        identity = singles.tile([P, P], F32, tag="identity", name="identity")
        make_identity(nc, identity)
        quarter_f = singles.tile([P, P], F32, tag="quarter_f", name="quarter_f")
        nc.vector.memset(quarter_f, 0.25)
        quarter = singles.tile([P, P], F32R, tag="quarter", name="quarter")
        nc.vector.tensor_copy(quarter, quarter_f)

        bts = [
            persist.tile([P, n], F32R, tag=f"bt{k}", name=f"bt{k}")
            for k in range(KC)
        ]
        sqb = persist.tile([P, n], F32, tag="sqb", name="sqb")

        # ---- Phase 0: load B, build Bt = -2*B^T ----
        for t in range(NB):
            b_nat = natp.tile([P, d], F32, tag="nat", name="b_nat")
            nc.sync.dma_start(out=b_nat, in_=b[t * P:(t + 1) * P, :])
            for k in range(KC):
                pt = tpp.tile([P, P], F32, tag="tp", name="pt")
                nc.tensor.transpose(pt, b_nat[:, k * P:(k + 1) * P], identity)
                nc.vector.tensor_scalar_mul(bts[k][:, t * P:(t + 1) * P], pt, -2.0)

        # ---- Phase 0b: sqb = ||B_j||^2 broadcast to all partitions ----
        for s in range(NS):
            nsl = slice(s * FD, (s + 1) * FD)
            ps = mmp.tile([P, FD], F32, tag="mm", name="ps_sqb")
            for k in range(KC):
                bsq = tmpp.tile([P, FD], F32R, tag="bsq", name="bsq")
                nc.vector.tensor_mul(bsq, bts[k][:, nsl], bts[k][:, nsl])
                nc.tensor.matmul(
                    ps, quarter, bsq,
                    start=(k == 0), stop=(k == KC - 1),
                )
            nc.vector.tensor_copy(sqb[:, nsl], ps)

        # ---- Phase 1: main loop over m tiles ----
        for m in range(MT):
            msl = slice(m * P, (m + 1) * P)
            a_nat = natp.tile([P, d], F32, tag="nat", name="a_nat")
            nc.sync.dma_start(out=a_nat, in_=a[msl, :])

            asq = tmpp.tile([P, d], F32, tag="asq", name="asq")
            nc.vector.tensor_mul(asq, a_nat, a_nat)
            sqa_col = sqap.tile([P, 1], F32, tag="sqa", name="sqa_col")
            nc.vector.tensor_reduce(sqa_col, asq, axis=AX.X, op=OP.add)

            ats = []
            for k in range(KC):
                pt = tpp.tile([P, P], F32, tag="tp", name="pt_a")
                nc.tensor.transpose(pt, a_nat[:, k * P:(k + 1) * P], identity)
                at = atp.tile([P, P], F32R, tag="at", name="at")
                nc.vector.tensor_copy(at, pt)
                ats.append(at)

            for h in range(OH):
                ostage = outp.tile([P, out_w], F32, tag="ostage", name="ostage")
                for sj in range(SPW):
                    s = h * SPW + sj
                    nsl = slice(s * FD, (s + 1) * FD)
                    ps = mmp.tile([P, FD], F32, tag="mm", name="ps_mm")
                    for k in range(KC):
                        nc.tensor.matmul(
                            ps, ats[k], bts[k][:, nsl],
                            start=(k == 0), stop=(k == KC - 1),
                        )
                    nc.vector.scalar_tensor_tensor(
                        out=ostage[:, sj * FD:(sj + 1) * FD],
                        in0=ps, scalar=sqa_col, in1=sqb[:, nsl],
                        op0=OP.add, op1=OP.add,
                    )
                nc.sync.dma_start(
                    out=o[msl, h * out_w:(h + 1) * out_w], in_=ostage
                )
    nc.finalize()
    return nc


_CACHE = {}


def _get_nc():
    if "nc" not in _CACHE:
        _CACHE["nc"] = build()
    return _CACHE["nc"]


def run(mat_1, mat_2, trace=False):
    from concourse.bass_utils import run_bass_kernel_spmd

    a = np.ascontiguousarray(np.asarray(mat_1, dtype=np.float32))
    b = np.ascontiguousarray(np.asarray(mat_2, dtype=np.float32))
    assert a.shape == (M_FULL, D_FULL) and b.shape == (N_FULL, D_FULL)
    m_sh = M_FULL // N_CORES
    nc = _get_nc()
    in_maps = [
        {"a": a[c * m_sh:(c + 1) * m_sh], "b": b} for c in range(N_CORES)
    ]
    res = run_bass_kernel_spmd(nc, in_maps, core_ids=list(range(N_CORES)), trace=trace)
    out = np.concatenate([r["out"] for r in res.results], axis=0)
    return out, res


def kernel(mat_1, mat_2):
    return run(mat_1, mat_2)[0]
